# revision 1
# baseline (speedup 1.0000x reference)
"""Grouped-experts SwiGLU MoE kernel for Trainium2 (8 NeuronCores).

Expert-parallel sharding: core e owns expert e's weights and its contiguous
token group (m_sizes gives T//E = 2048 tokens per expert). No collectives —
routing/scatter/gather happens on the host, each core runs an identical
single-core program on its own shard.

Per-core math: out = (silu(x_e @ w1_e) * (x_e @ w3_e)) @ w2_e
  x_e [2048, 2048], w1/w3 [2048, 1024], w2 [1024, 2048].

Device strategy (all matmul operands bf16, f32 PSUM accumulation —
rel(absmax) ≈ 4e-3 vs the f32 reference, well inside the 2e-2 gate):
  phase 1 (up+gate):  stationary = w1/w3 128x128 tiles, moving = xT tiles
      (pre-transposed on host so D is the partition/contraction axis).
      PSUM accumulates over D; SwiGLU evac (ACT silu + DVE mul) writes the
      intermediate zT [H, M] as bf16.
  phase 2 (down):     stationary = zT 128x128 tiles, moving = w2 tiles
      (resident in SBUF). PSUM accumulates over H; DVE copies to SBUF as
      bf16 (2x DVE throughput, half the store traffic) and DMA stores
      out [M, D] bf16 in natural orientation; the host upcasts to f32.

Scheduling notes (from perfetto traces):
  - bf16 operands run the PE at the full 1 cycle/row rate (~216ns per
    512-row matmul); f32r stationary tiles cost ~11ns/matmul extra.
  - The weight stream is issued on the ACT HWDGE queue while x / w2 / out
    use the SP queue, so the first w1 tile lands in parallel with the first
    xT chunk instead of queueing behind 4 of them (saves ~8us of startup).
  - Half 0's first two h-iterations are fused (8 matmuls per x-chunk): the
    cold DMA subsystem only sustains ~half the steady chunk rate, and
    halving the demand during the first ~25us removes the startup feed
    stalls that would otherwise also reset the PE's DVFS ramp.
  - w2's resident load is issued after the fused pair so phase 2 (at
    ~120us) never waits on it and it stays clear of the critical window.
  - Six dummy matmuls on a zeroed tile run the PE's DVFS ramp (~0.85 ->
    1.2 -> 2.37 GHz, ~3.4us of busy time) inside the startup DMA window,
    and the first x chunk / weight tile are split so the opening matmul
    waits on the smallest possible transfers.
  - The last token-block's output is stored as four 512-column DMAs so the
    final evac/DMA tail is shorter.
Tokens are processed in two halves of 1024 so the 4 PSUM accumulator banks
(2 for u, 2 for g) can ping-pong across h-iterations (bufs=2 -> 8 banks),
keeping the matmul stream free of evac stalls.
"""

import numpy as np
import ml_dtypes

E, T, D, H = 8, 16384, 2048, 1024
M = T // E            # tokens per expert
P = 128
DC = D // P           # 16 contraction chunks (phase 1)
HC = H // P           # 8 contraction chunks (phase 2)
NHALF = 2
MH = M // NHALF       # 1024 tokens per half
NMOV = 512            # moving free dim / PSUM bank width (f32)
G = 4                 # d-chunks per weight-stream DMA (128KB bf16 transfers)

_CACHE = {}
LAST_RESULTS = None   # for test harnesses that want the profile


def _build_program():
    import concourse.bacc as bacc
    import concourse.bass as bass
    import concourse.mybir as mybir
    import concourse.tile as tile

    f32 = mybir.dt.float32
    bf16 = mybir.dt.bfloat16
    SILU = mybir.ActivationFunctionType.Silu

    nc = bacc.Bacc("TRN2", target_bir_lowering=False, debug=False)

    xT = nc.dram_tensor("xT", [D, M], bf16, kind="ExternalInput")
    w1r = nc.dram_tensor("w1r", [HC, DC // G, P, G, P], bf16, kind="ExternalInput")
    w3r = nc.dram_tensor("w3r", [HC, DC // G, P, G, P], bf16, kind="ExternalInput")
    w2r = nc.dram_tensor("w2r", [HC, P, D], bf16, kind="ExternalInput")
    out = nc.dram_tensor("out", [M, D], bf16, kind="ExternalOutput")

    xT_t = xT.rearrange("(c p) m -> p c m", p=P)  # [P, DC, M]

    with tile.TileContext(nc) as tc:
        with (
            tc.tile_pool(name="xp", bufs=1) as xp,
            tc.tile_pool(name="w2p", bufs=1) as w2p,
            tc.tile_pool(name="zp", bufs=1) as zp,
            tc.tile_pool(name="wp", bufs=6) as wp,
            tc.tile_pool(name="op", bufs=2) as op,
            tc.tile_pool(name="sp", bufs=3) as sp,
            tc.tile_pool(name="wub", bufs=1) as wub,
            tc.tile_pool(name="ps", bufs=2, space=bass.MemorySpace.PSUM) as ps,
        ):
            w2t = w2p.tile([P, HC, D], bf16, tag="w2")

            # PE warm-up: the tensor engine DVFS-ramps (~0.85 -> 1.2 -> 2.37
            # GHz) over its first ~3.4us of busy time. Run the ramp on dummy
            # matmuls inside the startup DMA window (PE would be idle until
            # ~10.5us anyway) so the real stream starts at full clock. Sized
            # to end just as the first weight tile + x chunk land.
            wut = wub.tile([P, NMOV], bf16, tag="wu")
            nc.vector.memset(wut[:], 0)
            pwu = ps.tile([P, NMOV], f32, tag="p0", name="warm")
            NWARM = 6
            for i in range(NWARM):
                nc.tensor.matmul(
                    pwu[:], wut[:, 0:P], wut[:],
                    start=i == 0, stop=i == NWARM - 1,
                )

            for hf in range(NHALF):
                msl = slice(hf * MH, (hf + 1) * MH)
                xt = xp.tile([P, DC, MH], bf16, tag="xt")
                for c in range(DC):
                    # half-0 loads race the first weight tiles (on the other
                    # DGE queue); half-1 loads overlap half-0 phase 2
                    if hf == 0 and c == 0:
                        # first chunk in two pieces: the first matmul needs
                        # only 512 columns
                        nc.sync.dma_start(xt[:, 0, 0:NMOV], xT_t[:, 0, 0:NMOV])
                        nc.sync.dma_start(xt[:, 0, NMOV:MH], xT_t[:, 0, NMOV:MH])
                    else:
                        nc.sync.dma_start(xt[:, c, :], xT_t[:, c, msl])

                zt = zp.tile([P, HC, MH], bf16, tag="zt")

                # ---- phase 1: u = x@w1, g = x@w3, z = silu(u)*g ----
                # For half 0 the first two h-iterations are FUSED (8 matmuls
                # per x-chunk instead of 4): the startup x stream only
                # sustains ~half the steady-state chunk rate, and fusing
                # halves the demand exactly where the feed is coldest. The
                # fused pair uses all 8 PSUM banks, so h=2 briefly waits on
                # its evac — much cheaper than the feed stalls it removes.
                def p1_weights(h, cg, split=False):
                    w1t = wp.tile([P, G, P], bf16, tag="w1")
                    w3t = wp.tile([P, G, P], bf16, tag="w3")
                    if split:
                        # g=0 sub-tiles land first so the opening matmuls
                        # wait on 32KB transfers, not 128KB
                        nc.scalar.dma_start(w1t[:, 0:1, :], w1r[h, cg, :, 0:1, :])
                        nc.scalar.dma_start(w3t[:, 0:1, :], w3r[h, cg, :, 0:1, :])
                        nc.scalar.dma_start(w1t[:, 1:G, :], w1r[h, cg, :, 1:G, :])
                        nc.scalar.dma_start(w3t[:, 1:G, :], w3r[h, cg, :, 1:G, :])
                    else:
                        nc.scalar.dma_start(w1t[:], w1r[h, cg])
                        nc.scalar.dma_start(w3t[:], w3r[h, cg])
                    return w1t, w3t

                def p1_matmuls(h, pu, pg, w1t, w3t, g, c):
                    first, last = c == 0, c == DC - 1
                    for mi in range(MH // NMOV):
                        nc.tensor.matmul(
                            pu[mi][:], w1t[:, g, :],
                            xt[:, c, mi * NMOV:(mi + 1) * NMOV],
                            start=first, stop=last,
                        )
                    for mi in range(MH // NMOV):
                        nc.tensor.matmul(
                            pg[mi][:], w3t[:, g, :],
                            xt[:, c, mi * NMOV:(mi + 1) * NMOV],
                            start=first, stop=last,
                        )

                def p1_evac(h, pu, pg):
                    for mi in range(MH // NMOV):
                        st = sp.tile([P, NMOV], bf16, tag="st")
                        nc.scalar.activation(st[:], pu[mi][:], SILU)
                        nc.vector.tensor_mul(
                            zt[:, h, mi * NMOV:(mi + 1) * NMOV],
                            st[:], pg[mi][:],
                        )

                def p1_banks():
                    pu = [ps.tile([P, NMOV], f32, tag=f"p{i}", name=f"pu{i}") for i in range(2)]
                    pg = [ps.tile([P, NMOV], f32, tag=f"p{i + 2}", name=f"pg{i}") for i in range(2)]
                    return pu, pg

                if hf == 0:
                    acc = [p1_banks(), p1_banks()]
                    for cg in range(DC // G):
                        wts = [p1_weights(0, cg, split=cg == 0), p1_weights(1, cg)]
                        for g in range(G):
                            for h in range(2):
                                p1_matmuls(h, *acc[h], *wts[h], g, cg * G + g)
                    for h in range(2):
                        p1_evac(h, *acc[h])
                    # w2 resident for the whole kernel; issued once the
                    # critical startup window is past (needed only at ~120us)
                    for hh in range(HC):
                        nc.sync.dma_start(w2t[:, hh, :], w2r[hh])
                    h_rest = range(2, HC)
                else:
                    h_rest = range(HC)

                for h in h_rest:
                    pu, pg = p1_banks()
                    for cg in range(DC // G):
                        w1t, w3t = p1_weights(h, cg)
                        for g in range(G):
                            p1_matmuls(h, pu, pg, w1t, w3t, g, cg * G + g)
                    p1_evac(h, pu, pg)

                # ---- phase 2: out = z @ w2 ----
                for mi in range(MH // P):
                    po = [ps.tile([P, NMOV], f32, tag=f"p{dd}", name=f"po{dd}") for dd in range(4)]
                    for h in range(HC):
                        zst = zt[:, h, mi * P:(mi + 1) * P]
                        for dd in range(D // NMOV):
                            nc.tensor.matmul(
                                po[dd][:], zst,
                                w2t[:, h, dd * NMOV:(dd + 1) * NMOV],
                                start=h == 0, stop=h == HC - 1,
                            )
                    osb = op.tile([P, D], bf16, tag="o")
                    r0 = hf * MH + mi * P
                    if hf == NHALF - 1 and mi == MH // P - 1:
                        # last token block: PSUM evac split across ACT and DVE
                        # and stored per 512-col chunk so the final
                        # evac+DMA tail is as short as possible
                        for dd in range(D // NMOV):
                            dsl = slice(dd * NMOV, (dd + 1) * NMOV)
                            nc.vector.tensor_copy(osb[:, dsl], po[dd][:])
                            nc.sync.dma_start(out[r0:r0 + P, dsl], osb[:, dsl])
                    else:
                        for dd in range(D // NMOV):
                            nc.vector.tensor_copy(
                                osb[:, dd * NMOV:(dd + 1) * NMOV], po[dd][:]
                            )
                        nc.sync.dma_start(out[r0:r0 + P, :], osb[:])

    nc.compile()
    return nc


def _get_program():
    if "nc" not in _CACHE:
        _CACHE["nc"] = _build_program()
    return _CACHE["nc"]


def _prep_w13(w, g=G):
    # [D, H] -> [HC, DC//g, P, g, P]; element [h,cg,p,gg,m] = w[(cg*g+gg)*P+p, h*P+m]
    return np.ascontiguousarray(
        w.astype(ml_dtypes.bfloat16)
        .reshape(DC // g, g, P, HC, P)
        .transpose(3, 0, 2, 1, 4)
    )


def _numpy_fallback(x, w1, w2, w3, m_sizes):
    offs = np.concatenate([[0], np.cumsum(np.asarray(m_sizes, dtype=np.int64))])
    out = np.zeros((x.shape[0], w2.shape[2]), dtype=np.float32)
    for e in range(w1.shape[0]):
        xe = x[offs[e]:offs[e + 1]]
        u = xe @ w1[e]
        g = xe @ w3[e]
        z = (u / (1.0 + np.exp(-u))) * g
        out[offs[e]:offs[e + 1]] = z @ w2[e]
    return out


def kernel(x, w1, w2, w3, m_sizes, _trace=False, _trace_kwargs=None):
    global LAST_RESULTS
    x = np.ascontiguousarray(x, dtype=np.float32)
    w1 = np.ascontiguousarray(w1, dtype=np.float32)
    w2 = np.ascontiguousarray(w2, dtype=np.float32)
    w3 = np.ascontiguousarray(w3, dtype=np.float32)
    m = np.asarray(m_sizes, dtype=np.int64)

    expected = (
        x.shape == (T, D)
        and w1.shape == (E, D, H)
        and w2.shape == (E, H, D)
        and w3.shape == (E, D, H)
        and m.shape == (E,)
        and np.all(m == M)
    )
    if not expected:
        return _numpy_fallback(x, w1, w2, w3, m_sizes)

    from concourse.bass_utils import run_bass_kernel_spmd

    nc = _get_program()
    in_maps = []
    for e in range(E):
        in_maps.append({
            "xT": np.ascontiguousarray(
                x[e * M:(e + 1) * M].astype(ml_dtypes.bfloat16).T
            ),
            "w1r": _prep_w13(w1[e]),
            "w3r": _prep_w13(w3[e]),
            "w2r": np.ascontiguousarray(
                w2[e].astype(ml_dtypes.bfloat16).reshape(HC, P, D)
            ),
        })

    res = run_bass_kernel_spmd(
        nc, in_maps, core_ids=list(range(E)),
        trace=_trace, **(_trace_kwargs or {}),
    )
    LAST_RESULTS = res
    return np.concatenate(
        [np.asarray(r["out"]) for r in res.results], axis=0
    ).astype(np.float32)

